# revision 1
# baseline (speedup 1.0000x reference)
"""Expert-choice MoE router kernel for Trainium2 (8 NeuronCores).

Problem (B=4, T=8192, D=512, E=8, H=2048, C=1024):
  scores = x @ Wg                         (B, T, E)
  w      = softmax(scores^T over T)       (B, E, T)
  top-C tokens per (b, e) by w            (expert choice)
  y_e    = gelu(x[sel] @ W1[e]) @ W2[e] * w[sel]
  out    = scatter_add(y_e) / max(scatter_add(w[sel]), 1e-8)

Sharding: expert-parallel, one expert per core (E == n_cores == 8).
  - scores: each core computes partial fp32 scores for its 1/8 token shard
    (from a host-pretransposed x shard); AllToAll redistributes so core e
    holds expert e's full-T scores.
  - top-C selection: fixed-round threshold bisection on fp32 scores
    (vectorized over the 4 batch rows), then GPSIMD sparse_gather compacts
    indices/values (selection order is output-invariant).
  - relayout 16->128 partitions goes through a PE transpose plus a 0/1
    selection matmul (exact for integer-valued f32).
  - FFN: gpsimd dma_gather(transpose=True) fetches selected tokens in bf16
    directly in [d-partition, token-free] layout; two bf16 matmul chains
    with exact gelu between; weighted outputs + gate value are packed into
    (D+8)-f32 rows and indirect-DMA-scattered into a per-core dense
    (B*T, D+8) buffer (indices within one expert are unique).
  - combine: ReduceScatter(add) sums the dense buffers across cores; each
    core normalizes its 1/8 row shard; host concatenates.
"""

import sys
from dataclasses import dataclass

sys.path.insert(0, "/opt/trn_rl_repo")

import numpy as np
import ml_dtypes

import concourse.bass as bass  # noqa: F401
import concourse.mybir as mybir
import concourse.tile as tile
from concourse import bacc
from concourse.bass import IndirectOffsetOnAxis
from concourse.bass_utils import run_bass_kernel_spmd

F32 = mybir.dt.float32
BF16 = mybir.dt.bfloat16
I16 = mybir.dt.int16
I32 = mybir.dt.int32
U32 = mybir.dt.uint32
AF = mybir.ActivationFunctionType
ALU = mybir.AluOpType

NCORES = 8


@dataclass(frozen=True)
class Cfg:
    B: int = 4
    T: int = 8192
    D: int = 512
    E: int = 8
    H: int = 2048
    C: int = 1024
    nrounds: int = 40
    act: str = "Gelu"
    stage: int = 3   # 1=thru relayout, 2=+ffn/scatter, 3=full
    sub: str = ""    # stage-2 sub-gate: gather|mm1|mm2|scatter

    @property
    def BT(self):
        return self.B * self.T

    @property
    def TSH(self):
        return self.BT // NCORES

    @property
    def ROW(self):
        return self.D + 8

    @property
    def DC(self):
        return self.D // 128

    @property
    def HC(self):
        return self.H // 128

    @property
    def PPB(self):
        return 128 // self.B          # partitions per batch (bisect layout)

    @property
    def TPP(self):
        return self.T // self.PPB     # tokens per partition (bisect layout)

    @property
    def RPB(self):
        return NCORES // self.B       # a2a rows (shards) per batch

    @property
    def QL(self):
        return self.T // self.RPB // 16   # w16 columns per (b, shard-row)

    @property
    def CF(self):
        return self.C // 16           # compacted columns

    @property
    def CS(self):
        return self.C // 128          # c-subtiles per batch


FULL = Cfg()


def build_nc(cfg: Cfg = FULL):
    B, T, D, E, H, C = cfg.B, cfg.T, cfg.D, cfg.E, cfg.H, cfg.C
    BT, TSH, ROW, DC, HC = cfg.BT, cfg.TSH, cfg.ROW, cfg.DC, cfg.HC
    PPB, TPP, RPB, QL = cfg.PPB, cfg.TPP, cfg.RPB, cfg.QL
    CF, CS = cfg.CF, cfg.CS
    TB16 = T // 16          # w16 columns per batch
    NT0 = min(512, C)
    PPR = PPB // RPB        # partitions per a2a row in w128 layout

    nc = bacc.Bacc("TRN2", target_bir_lowering=False, debug=False,
                   num_devices=NCORES)

    # ---- I/O ----
    x_bf = nc.dram_tensor("x_bf", [BT, D], BF16, kind="ExternalInput")
    xt_sh = nc.dram_tensor("xt_sh", [D, TSH], F32, kind="ExternalInput")
    wg_d = nc.dram_tensor("wg", [D, E], F32, kind="ExternalInput")
    w1_d = nc.dram_tensor("w1", [D, H], BF16, kind="ExternalInput")
    w2_d = nc.dram_tensor("w2", [H, D], BF16, kind="ExternalInput")
    # host-precomputed constants
    iotap1_d = nc.dram_tensor("iotap1", [16, B * TB16], F32, kind="ExternalInput")
    e1_d = nc.dram_tensor("e1", [128, B], F32, kind="ExternalInput")
    e2_d = nc.dram_tensor("e2", [B, 128], F32, kind="ExternalInput")
    o416_d = nc.dram_tensor("o416", [B, 16], F32, kind="ExternalInput")
    id4_d = nc.dram_tensor("id4", [B, B], I32, kind="ExternalInput")
    id16_d = nc.dram_tensor("id16", [16, 16], F32, kind="ExternalInput")
    idbf_d = nc.dram_tensor("idbf", [128, 128], BF16, kind="ExternalInput")
    o16_d = nc.dram_tensor("o16", [16, 1], F32, kind="ExternalInput")
    mk_d = nc.dram_tensor("mk", [CF, 128], F32, kind="ExternalInput")
    rsel_d = nc.dram_tensor("rsel", [CF, CS], F32, kind="ExternalInput")

    out_sh = nc.dram_tensor("out_sh", [TSH, D], F32, kind="ExternalOutput")
    nf_out = nc.dram_tensor("nf_out", [B, 2], U32, kind="ExternalOutput")
    dbg = {}
    if cfg.stage < 3:
        dbg["lo"] = nc.dram_tensor("dbg_lo", [B, 1], F32, kind="ExternalOutput")
        dbg["idx"] = nc.dram_tensor("dbg_idx", [B, 128, CS], I32,
                                    kind="ExternalOutput")
        dbg["val"] = nc.dram_tensor("dbg_val", [B, 128, CS], F32,
                                    kind="ExternalOutput")
        dbg["a2a"] = nc.dram_tensor("dbg_a2a", [E, TSH], F32,
                                    kind="ExternalOutput")
    if cfg.stage == 2:
        dbg["dense"] = nc.dram_tensor("dbg_dense", [BT, ROW], F32,
                                      kind="ExternalOutput")
        if cfg.sub in ("gather", "mm1", "mm2"):
            dbg["selT"] = nc.dram_tensor("dbg_selT", [128, DC * C], BF16,
                                         kind="ExternalOutput")
        if cfg.sub in ("mm1", "mm2"):
            dbg["hT"] = nc.dram_tensor("dbg_hT", [128, HC * NT0], BF16,
                                       kind="ExternalOutput")
        if cfg.sub == "mm2":
            dbg["pk"] = nc.dram_tensor("dbg_pk", [128, CS * ROW], F32,
                                       kind="ExternalOutput")

    # ---- internal DRAM ----
    a2a_in = nc.dram_tensor("a2a_in", [E, TSH], F32)
    a2a_out = nc.dram_tensor("a2a_out", [E, TSH], F32)
    dense = nc.dram_tensor("dense", [BT, ROW], F32)
    rs_out = nc.dram_tensor("rs_out", [TSH, ROW], F32)

    with tile.TileContext(nc) as tc:
        with (
            tc.tile_pool(name="const", bufs=1) as cp,
            tc.tile_pool(name="sc", bufs=2) as scp,
            tc.tile_pool(name="bis", bufs=1) as bp,
            tc.tile_pool(name="ffn", bufs=2) as fp,
            tc.tile_pool(name="pk", bufs=2) as pkp,
            tc.tile_pool(name="norm", bufs=3) as np_,
            tc.tile_pool(name="pmm", bufs=2, space="PSUM") as pmm,
            tc.tile_pool(name="pps", bufs=3, space="PSUM") as pps,
        ):
            # ---------- phase 0: zero the dense accumulator ----------
            ZF = 2 * ROW            # flat f32 per partition per zero-DMA
            zt = cp.tile([128, ZF], F32, tag="zt")
            nc.vector.memset(zt[:], 0.0)
            dense_z = dense.ap().rearrange("(j p zr) r -> j p (zr r)", p=128, zr=2)
            for j in range(BT // 256):
                nc.sync.dma_start(dense_z[j], zt[:])

            # ---------- load constants / weights ----------
            wg_sb = cp.tile([128, DC, E], F32, tag="wg_sb")
            nc.sync.dma_start(wg_sb[:], wg_d.ap().rearrange("(c p) e -> p c e", p=128))
            w1_sb = cp.tile([128, DC, H], BF16, tag="w1_sb")
            nc.sync.dma_start(w1_sb[:], w1_d.ap().rearrange("(c p) h -> p c h", p=128))
            w2_sb = cp.tile([128, HC, D], BF16, tag="w2_sb")
            nc.sync.dma_start(w2_sb[:], w2_d.ap().rearrange("(c p) d -> p c d", p=128))
            iotap1 = cp.tile([16, B * TB16], F32, tag="iotap1")
            nc.sync.dma_start(iotap1[:], iotap1_d.ap())
            e1s = cp.tile([128, B], F32, tag="e1s")
            nc.sync.dma_start(e1s[:], e1_d.ap())
            e2s = cp.tile([B, 128], F32, tag="e2s")
            nc.sync.dma_start(e2s[:], e2_d.ap())
            o416 = cp.tile([B, 16], F32, tag="o416")
            nc.sync.dma_start(o416[:], o416_d.ap())
            id4s = cp.tile([B, B], I32, tag="id4s")
            nc.sync.dma_start(id4s[:], id4_d.ap())
            id16s = cp.tile([16, 16], F32, tag="id16s")
            nc.sync.dma_start(id16s[:], id16_d.ap())
            idbfs = cp.tile([128, 128], BF16, tag="idbfs")
            nc.sync.dma_start(idbfs[:], idbf_d.ap())
            o16s = cp.tile([16, 1], F32, tag="o16s")
            nc.sync.dma_start(o16s[:], o16_d.ap())
            mks = cp.tile([CF, 128], F32, tag="mks")
            nc.sync.dma_start(mks[:], mk_d.ap())
            rsels = cp.tile([CF, CS], F32, tag="rsels")
            nc.sync.dma_start(rsels[:], rsel_d.ap())

            # ---------- phase 1: partial scores for my token shard ----------
            # scores^T partial: (E, TSH) = Wg^T @ x_shard^T
            for nt in range(TSH // 512):
                xt_t = scp.tile([128, DC, 512], F32, tag="xt")
                nc.sync.dma_start(
                    xt_t[:],
                    xt_sh.ap().rearrange("(c p) t -> p c t", p=128)[
                        :, :, nt * 512:(nt + 1) * 512],
                )
                ps_sc = pps.tile([E, 512], F32, tag="sp")
                for dc in range(DC):
                    nc.tensor.matmul(ps_sc[:], lhsT=wg_sb[:, dc, :],
                                     rhs=xt_t[:, dc, :],
                                     start=(dc == 0), stop=(dc == DC - 1))
                sc_sb = scp.tile([E, 512], F32, tag="scsb")
                nc.vector.tensor_copy(sc_sb[:], ps_sc[:])
                nc.sync.dma_start(a2a_in[:, nt * 512:(nt + 1) * 512], sc_sb[:])

            # ---------- phase 2: AllToAll -> my expert's full-T scores ----
            nc.gpsimd.collective_compute(
                "AllToAll", ALU.bypass, replica_groups=[list(range(NCORES))],
                ins=[a2a_in.ap()], outs=[a2a_out.ap()],
            )

            # w128: (128, TPP); partition b*PPB + h*PPR + l holds tokens
            #   [(h*PPR+l)*TPP, ...) of batch b  (contiguous per-row loads)
            w128 = cp.tile([128, TPP], F32, tag="w128")
            for r in range(E):
                nc.sync.dma_start(
                    w128[r * PPR:(r + 1) * PPR, :],
                    a2a_out.ap()[r].rearrange("(l f) -> l f", l=PPR))
            # w16: (16, B*TB16); [s, b*TB16 + q*QL + j]
            #   = scores[b, q*(T/RPB) + s*QL + j]
            w16 = cp.tile([16, B * TB16], F32, tag="w16")
            for r in range(E):
                b, q = divmod(r, RPB)
                nc.sync.dma_start(
                    w16[:, b * TB16 + q * QL: b * TB16 + (q + 1) * QL],
                    a2a_out.ap()[r].rearrange("(s j) -> s j", s=16))

            # ---------- phase 3: softmax pieces (exp + row sums) ----------
            exp16 = cp.tile([16, B * TB16], F32, tag="exp16")
            parts16 = bp.tile([16, B], F32, tag="parts16")
            for b in range(B):
                sl = slice(b * TB16, (b + 1) * TB16)
                nc.scalar.activation(exp16[:, sl], w16[:, sl], AF.Exp,
                                     accum_out=parts16[:, b:b + 1])
            ps4 = pps.tile([B, 1], F32, tag="sp")
            nc.tensor.matmul(ps4[:], lhsT=parts16[:], rhs=o16s[:],
                             start=True, stop=True)
            recip4 = bp.tile([B, 1], F32, tag="recip4")
            nc.vector.reciprocal(recip4[:], ps4[:])
            diagr = bp.tile([B, B], F32, tag="diagr")
            nc.vector.memset(diagr[:], 0.0)
            nc.vector.copy_predicated(diagr[:], id4s[:],
                                      recip4[:, 0:1].to_broadcast([B, B]))
            psr16 = pps.tile([16, B], F32, tag="sp")
            nc.tensor.matmul(psr16[:], lhsT=o416[:], rhs=diagr[:],
                             start=True, stop=True)
            recip16 = cp.tile([16, B], F32, tag="recip16")
            nc.vector.tensor_copy(recip16[:], psr16[:])

            # ---------- phase 4: threshold bisection (top-C cut) -------
            lo4 = bp.tile([B, 1], F32, tag="lo4")
            hi4 = bp.tile([B, 1], F32, tag="hi4")
            nc.vector.memset(lo4[:], -20.0)
            nc.vector.memset(hi4[:], 20.0)
            mid4 = bp.tile([B, 1], F32, tag="mid4")
            sel4 = bp.tile([B, 1], I32, tag="sel4")
            seli4 = bp.tile([B, 1], I32, tag="seli4")
            midbc = bp.tile([128, 1], F32, tag="midbc")
            cnt128 = bp.tile([128, 1], F32, tag="cnt128")
            msk = bp.tile([128, TPP], F32, tag="msk")
            for _ in range(cfg.nrounds):
                nc.vector.tensor_add(mid4[:], lo4[:], hi4[:])
                nc.vector.tensor_scalar_mul(mid4[:], mid4[:], 0.5)
                pmb = pps.tile([128, 1], F32, tag="sp")
                nc.tensor.matmul(pmb[:], lhsT=e2s[:], rhs=mid4[:],
                                 start=True, stop=True)
                nc.vector.tensor_copy(midbc[:], pmb[:])
                nc.vector.tensor_scalar(msk[:], w128[:], midbc[:, 0:1], None,
                                        op0=ALU.is_ge, op1=ALU.add,
                                        accum_out=cnt128[:, 0:1])
                pc4 = pps.tile([B, 1], F32, tag="sp")
                nc.tensor.matmul(pc4[:], lhsT=e1s[:], rhs=cnt128[:],
                                 start=True, stop=True)
                nc.vector.tensor_scalar(sel4[:], pc4[:], float(C) - 0.5, None,
                                        op0=ALU.is_ge)
                nc.vector.tensor_scalar(seli4[:], pc4[:], float(C) - 0.5, None,
                                        op0=ALU.is_lt)
                nc.vector.copy_predicated(lo4[:], sel4[:], mid4[:])
                nc.vector.copy_predicated(hi4[:], seli4[:], mid4[:])

            # tau16[:, b] = lo4[b] replicated over 16 partitions
            diagt = bp.tile([B, B], F32, tag="diagt")
            nc.vector.memset(diagt[:], 0.0)
            nc.vector.copy_predicated(diagt[:], id4s[:],
                                      lo4[:, 0:1].to_broadcast([B, B]))
            pst16 = pps.tile([16, B], F32, tag="sp")
            nc.tensor.matmul(pst16[:], lhsT=o416[:], rhs=diagt[:],
                             start=True, stop=True)
            tau16 = cp.tile([16, B], F32, tag="tau16")
            nc.vector.tensor_copy(tau16[:], pst16[:])
            if cfg.stage < 3:
                nc.sync.dma_start(dbg["lo"].ap(), lo4[:])
                for r in range(E):
                    dbt = np_.tile([128, TSH // 128], F32, tag="dbt")
                    nc.sync.dma_start(
                        dbt[:], a2a_out.ap()[r].rearrange("(p c) -> p c", p=128))
                    nc.sync.dma_start(
                        dbg["a2a"].ap()[r].rearrange("(p c) -> p c", p=128),
                        dbt[:])

            # ---------- phase 5: compaction + 16->128 relayout ----------
            idx32s = []     # (128, CS) int32 global row index
            val128s = []    # (128, CS) f32 gate vals
            for b in range(B):
                sl = slice(b * TB16, (b + 1) * TB16)
                mask16 = bp.tile([16, TB16], F32, tag="mask16")
                nc.vector.tensor_scalar(mask16[:], w16[:, sl], tau16[:, b:b + 1],
                                        None, op0=ALU.is_ge)
                candi = bp.tile([16, TB16], F32, tag="candi")
                nc.vector.tensor_tensor(candi[:], mask16[:], iotap1[:, sl],
                                        op=ALU.mult)
                nc.vector.tensor_scalar_add(candi[:], candi[:], -1.0)
                candv = bp.tile([16, TB16], F32, tag="candv")
                nc.vector.tensor_tensor(candv[:], mask16[:], exp16[:, sl],
                                        op=ALU.mult)
                nc.vector.tensor_scalar_add(mask16[:], mask16[:], -1.0)
                nc.vector.tensor_tensor(candv[:], candv[:], mask16[:],
                                        op=ALU.add)

                ci = bp.tile([16, CF + 16], F32, tag=f"ci{b}")
                nfi = bp.tile([1, 1], U32, tag=f"nfi{b}")
                nc.gpsimd.sparse_gather(ci[:], candi[:], num_found=nfi[:])
                cv = bp.tile([16, CF + 16], F32, tag=f"cv{b}")
                nfv = bp.tile([1, 1], U32, tag=f"nfv{b}")
                nc.gpsimd.sparse_gather(cv[:], candv[:], num_found=nfv[:])
                nc.sync.dma_start(nf_out.ap()[b:b + 1, 0:1], nfi[:, :])
                nc.sync.dma_start(nf_out.ap()[b:b + 1, 1:2], nfv[:, :])

                # vals = exp * (1/rowsum)
                nc.vector.tensor_scalar(cv[:, :CF], cv[:, :CF],
                                        recip16[:, b:b + 1], None, op0=ALU.mult)
                # global row index = t + b*T (fits int16/f32-exact: max 32767)
                nc.vector.tensor_scalar_add(ci[:, :CF], ci[:, :CF],
                                            float(b * T))

                # 16->128 relayout: transpose (16, CF) -> (CF, 16), replicate
                # columns x8, mask by (f%8 == g), then one selection matmul.
                pti = pps.tile([CF, 16], F32, tag="sp")
                nc.tensor.transpose(pti[:], ci[:, :CF], id16s[:])
                cit = bp.tile([CF, 16], F32, tag="cit")
                nc.vector.tensor_copy(cit[:], pti[:])
                ptv = pps.tile([CF, 16], F32, tag="sp")
                nc.tensor.transpose(ptv[:], cv[:, :CF], id16s[:])
                cvt = bp.tile([CF, 16], F32, tag="cvt")
                nc.vector.tensor_copy(cvt[:], ptv[:])

                cmi = bp.tile([CF, 128], F32, tag="cmi")
                nc.vector.tensor_tensor(
                    cmi[:].rearrange("f (g s) -> f g s", g=8),
                    cit[:, None, :].to_broadcast([CF, 8, 16]),
                    mks[:].rearrange("f (g s) -> f g s", g=8),
                    op=ALU.mult)
                cmv = bp.tile([CF, 128], F32, tag="cmv")
                nc.vector.tensor_tensor(
                    cmv[:].rearrange("f (g s) -> f g s", g=8),
                    cvt[:, None, :].to_broadcast([CF, 8, 16]),
                    mks[:].rearrange("f (g s) -> f g s", g=8),
                    op=ALU.mult)

                pri = pps.tile([128, CS], F32, tag="sp")
                nc.tensor.matmul(pri[:], lhsT=cmi[:], rhs=rsels[:],
                                 start=True, stop=True)
                idx32 = cp.tile([128, CS], I32, name=f"idx32_{b}",
                                tag=f"idx32_{b}")
                nc.vector.tensor_copy(idx32[:], pri[:])
                prv = pps.tile([128, CS], F32, tag="sp")
                nc.tensor.matmul(prv[:], lhsT=cmv[:], rhs=rsels[:],
                                 start=True, stop=True)
                val128 = cp.tile([128, CS], F32, name=f"val128_{b}",
                                 tag=f"val128_{b}")
                nc.vector.tensor_copy(val128[:], prv[:])

                if cfg.stage < 3:
                    nc.sync.dma_start(dbg["idx"].ap()[b], idx32[:])
                    nc.sync.dma_start(dbg["val"].ap()[b], val128[:])
                idx32s.append(idx32)
                val128s.append(val128)

            # ---------- phase 6: per-batch FFN + one scatter ----------
            NT = NT0            # matmul moving-dim tile
            nb6 = B if cfg.stage >= 2 else 0
            if cfg.sub and cfg.sub != "scatter":
                nb6 = min(nb6, 1)
            for b in range(nb6):
                selTM = fp.tile([128, CS, D], BF16, tag="selTM", bufs=1)
                for cs in range(CS):
                    nc.gpsimd.indirect_dma_start(
                        out=selTM[:, cs, :],
                        out_offset=None,
                        in_=x_bf.ap(),
                        in_offset=IndirectOffsetOnAxis(
                            ap=idx32s[b][:, cs:cs + 1], axis=0))
                selT = fp.tile([128, DC, C], BF16, tag="selT")
                for cs in range(CS):
                    for dc in range(DC):
                        ptp = pps.tile([128, 128], BF16, tag="tp")
                        nc.tensor.transpose(
                            ptp[:], selTM[:, cs, dc * 128:(dc + 1) * 128],
                            idbfs[:])
                        nc.vector.tensor_copy(
                            selT[:, dc, cs * 128:(cs + 1) * 128], ptp[:])
                if cfg.sub and b == 0 and "selT" in dbg:
                    nc.sync.dma_start(
                        dbg["selT"].ap().rearrange("p (c x) -> p c x", c=DC),
                        selT[:])
                if cfg.sub == "gather":
                    continue
                pk = pkp.tile([128, CS, ROW], F32, tag="pk", bufs=1)
                nc.vector.memset(pk[:], 0.0)
                nct = C // NT if cfg.sub != "mm1" else 1
                for ct in range(nct):
                    csl = slice(ct * NT, (ct + 1) * NT)
                    hT = fp.tile([128, HC, NT], BF16, tag="hT")
                    for ht in range(HC):
                        psh = pmm.tile([128, NT], F32, tag="mm")
                        for dc in range(DC):
                            nc.tensor.matmul(
                                psh[:],
                                lhsT=w1_sb[:, dc, ht * 128:(ht + 1) * 128],
                                rhs=selT[:, dc, csl],
                                start=(dc == 0), stop=(dc == DC - 1))
                        nc.scalar.activation(hT[:, ht, :], psh[:],
                                             getattr(AF, cfg.act))
                    if cfg.sub in ("mm1", "mm2") and b == 0 and ct == 0:
                        nc.sync.dma_start(
                            dbg["hT"].ap().rearrange("p (c x) -> p c x", c=HC),
                            hT[:])
                    if cfg.sub == "mm1":
                        continue
                    for cl in range(NT // 128):
                        cs = ct * (NT // 128) + cl
                        pso = pmm.tile([128, D], F32, tag="mm")
                        for hc in range(HC):
                            nc.tensor.matmul(
                                pso[:],
                                lhsT=hT[:, hc, cl * 128:(cl + 1) * 128],
                                rhs=w2_sb[:, hc, :],
                                start=(hc == 0), stop=(hc == HC - 1))
                        nc.vector.tensor_scalar(
                            pk[:, cs, :D], pso[:],
                            val128s[b][:, cs:cs + 1], None, op0=ALU.mult)
                        nc.vector.tensor_copy(pk[:, cs, D:D + 1],
                                              val128s[b][:, cs:cs + 1])
                if cfg.sub == "mm2":
                    if b == 0:
                        nc.sync.dma_start(
                            dbg["pk"].ap().rearrange("p (c x) -> p c x", c=CS),
                            pk[:])
                    continue
                for cs in range(CS):
                    nc.gpsimd.indirect_dma_start(
                        out=dense.ap(),
                        out_offset=IndirectOffsetOnAxis(
                            ap=idx32s[b][:, cs:cs + 1], axis=0),
                        in_=pk[:, cs, :],
                        in_offset=None,
                        bounds_check=BT - 1,
                        oob_is_err=False)

            # ---------- phase 7: ReduceScatter + normalize ----------
            if cfg.stage == 2:
                dzi = dense.ap().rearrange("(j p) r -> j p r", p=128)
                dzo = dbg["dense"].ap().rearrange("(j p) r -> j p r", p=128)
                for j in range(BT // 128):
                    dbd = np_.tile([128, ROW], F32, tag="dbd")
                    nc.sync.dma_start(dbd[:], dzi[j])
                    nc.sync.dma_start(dzo[j], dbd[:])
            if cfg.stage >= 3:
                nc.gpsimd.collective_compute(
                    "ReduceScatter", ALU.add, replica_groups=[list(range(NCORES))],
                    ins=[dense.ap()], outs=[rs_out.ap()],
                )
            for j in range(TSH // 128 if cfg.stage >= 3 else 0):
                rsl = slice(j * 128, (j + 1) * 128)
                ld = np_.tile([128, D + 1], F32, tag="ld")
                nc.sync.dma_start(ld[:], rs_out.ap()[rsl, :D + 1])
                dn = np_.tile([128, 1], F32, tag="dn")
                nc.vector.tensor_scalar(dn[:], ld[:, D:D + 1], 1e-8, None,
                                        op0=ALU.max)
                rc = np_.tile([128, 1], F32, tag="rc")
                nc.vector.reciprocal(rc[:], dn[:])
                ot = np_.tile([128, D], F32, tag="ot")
                nc.vector.tensor_scalar(ot[:], ld[:, :D], rc[:, 0:1], None,
                                        op0=ALU.mult)
                nc.sync.dma_start(out_sh.ap()[rsl, :], ot[:])

    nc.compile()
    return nc


# ---------------------------------------------------------------------------
# host side
# ---------------------------------------------------------------------------

def host_consts(cfg: Cfg = FULL):
    B, T = cfg.B, cfg.T
    TB16, RPB, QL, CF, CS = T // 16, cfg.RPB, cfg.QL, cfg.CF, cfg.CS
    iotap1 = np.zeros((16, B * TB16), np.float32)
    for s in range(16):
        for q in range(RPB):
            j = np.arange(QL)
            t = q * (T // RPB) + s * QL + j
            for b in range(B):
                iotap1[s, b * TB16 + q * QL + j] = t + 1
    p = np.arange(128)
    e1 = (p[:, None] // cfg.PPB == np.arange(B)[None, :]).astype(np.float32)
    e2 = np.ascontiguousarray(e1.T)
    o416 = np.ones((B, 16), np.float32)
    id4 = np.eye(B, dtype=np.int32)
    id16 = np.eye(16, dtype=np.float32)
    idbf = np.eye(128).astype(ml_dtypes.bfloat16)
    o16 = np.ones((16, 1), np.float32)
    f = np.arange(CF)
    g = np.arange(8)
    mk = np.zeros((CF, 128), np.float32)
    mk.reshape(CF, 8, 16)[:, :, :] = (f[:, None] % 8 == g[None, :]).astype(
        np.float32)[:, :, None]
    rsel = (f[:, None] // 8 == np.arange(CS)[None, :]).astype(np.float32)
    return dict(iotap1=iotap1, e1=e1, e2=e2, o416=o416, id4=id4, id16=id16,
                idbf=idbf, o16=o16, mk=mk, rsel=rsel)


def make_in_maps(inputs, cfg: Cfg = FULL):
    x = np.asarray(inputs["x"], np.float32).reshape(cfg.BT, cfg.D)
    Wg = np.ascontiguousarray(np.asarray(inputs["Wg"], np.float32))
    W1 = np.asarray(inputs["W1"], np.float32)
    W2 = np.asarray(inputs["W2"], np.float32)
    consts = host_consts(cfg)
    x_bf = x.astype(ml_dtypes.bfloat16)
    in_maps = []
    for i in range(NCORES):
        m = dict(consts)
        m["x_bf"] = x_bf
        m["xt_sh"] = np.ascontiguousarray(x[i * cfg.TSH:(i + 1) * cfg.TSH].T)
        m["wg"] = Wg
        m["w1"] = np.ascontiguousarray(W1[i].astype(ml_dtypes.bfloat16))
        m["w2"] = np.ascontiguousarray(W2[i].astype(ml_dtypes.bfloat16))
        in_maps.append(m)
    return in_maps


def assemble_out(results, cfg: Cfg = FULL):
    nf = np.stack([np.asarray(results[i]["nf_out"]) for i in range(NCORES)])
    if not (nf == cfg.C).all():
        print(f"WARNING: sparse_gather num_found != {cfg.C}: {nf.tolist()}",
              file=sys.stderr)
    out = np.concatenate([results[i]["out_sh"] for i in range(NCORES)], 0)
    return np.ascontiguousarray(out.reshape(cfg.B, cfg.T, cfg.D), dtype=np.float32)


_NC_CACHE = {}


def get_nc():
    if "nc" not in _NC_CACHE:
        _NC_CACHE["nc"] = build_nc(FULL)
    return _NC_CACHE["nc"]


def kernel(**inputs):
    nc = get_nc()
    in_maps = make_in_maps(inputs, FULL)
    res = run_bass_kernel_spmd(nc, in_maps, core_ids=list(range(NCORES)),
                               **_NC_CACHE.get("run_kwargs", {}))
    _NC_CACHE["last_run"] = res
    return assemble_out(res.results, FULL)

